# revision 68
# baseline (speedup 1.0000x reference)
"""Multi-head attention (nn_Attention1D) on 8 Trainium2 NeuronCores.

Full inputs in, full output out.  Sharding: batch (2) x head-groups (4 heads
per core, E=256 e-columns).  Per-core pipeline (ACT exp stream is the
critical resource; everything else hides under it):

  QKV projections: compensated fp8 DoubleRow matmuls (3 terms:
      xh@wh + (xl*4)@(wh/4) + (xh/4)@(wl*4), weights pre-scaled by 64 into
      e4m3's normal range, rescaled in the bias-add) -> bf16-level accuracy
      at 1/4 the PE cost of bf16.  q/k stored bf16 [dk, s]; v stored
      bf16 [s, (h, dk|1)] with a ones column (softmax denominator for free).
  scores:   scoresT[sk, q] = kT.T @ qT per (head, sk-tile), fp32 PSUM.
  softmax:  ACT exp -> bf16; DVE multiply by softmask tile -> pT (bf16).
  PV:       flipped orientation: stationary = pT tile [k,q], moving =
            v [k, 65] -> xa[q, 64|denom] accumulated over sk (2x fewer
            streamed columns than the [dk, q] orientation).
  norm:     DVE reciprocal of the denom column + per-partition scalar mul.
  out-proj: PE-transpose xatt [q,e] -> xattT [e,q] via identity matmuls,
            then out[q, d] = xattT.T @ wo, copies on GPSIMD, bf16 out.
  Host sums the 4 per-core partials per batch and adds bo.
"""

import math
from collections import deque

import numpy as np

import concourse.bass as bass
import concourse.mybir as mybir
import concourse.tile as tile

F32 = mybir.dt.float32
BF16 = mybir.dt.bfloat16
F8 = mybir.dt.float8e4
DR = mybir.MatmulPerfMode.DoubleRow
EXP = mybir.ActivationFunctionType.Exp
MULT = mybir.AluOpType.mult
ADD = mybir.AluOpType.add

P = 128
WS = 64.0  # weight pre-scale into e4m3 normal range


def _split_multiwait(nc, max_waits=1):
    """This walrus build only accepts one sync wait per instruction; hoist
    extra waits onto NoOps inserted just before."""
    for bb in nc.main_func.blocks:
        new_insts = []
        for ins in bb.instructions:
            if ins.sync_info and len(ins.sync_info.on_wait) > max_waits:
                waits = list(ins.sync_info.on_wait)
                ins.sync_info.on_wait = waits[:max_waits]
                for i, w in enumerate(waits[max_waits:]):
                    nop = mybir.InstNoOp(name=f"{ins.name}_ws{i}", ins=[], outs=[])
                    nop.engine = ins.engine
                    nop.sync_info = mybir.SyncInfo(on_wait=[w], on_update=[])
                    nc.register_instruction(nop)
                    new_insts.append(nop)
            new_insts.append(ins)
        bb.instructions = new_insts


def build_program(D=1024, S=2048, E=256, DK=64):
    H = E // DK          # 4 heads per core
    KE = E // P          # 2 e-tiles
    KT = D // 256        # 4 DoubleRow k-tiles (K=256 each)
    SK = S // P          # 16 sk-tiles
    CS = 512             # projection chunk (s columns)
    NCS = S // CS        # 4
    CQ = 1024            # attention q chunk
    NCQ = S // CQ        # 2
    QS = CQ // P         # 8 q-subtiles per chunk
    DK1 = DK + 1

    nc = bass.Bass()
    xq8 = nc.dram_tensor("xq8", [P, S // 512, 2, KT, 2, 512], F8, kind="ExternalInput")
    xk8 = nc.dram_tensor("xk8", [P, S // 512, 2, KT, 2, 512], F8, kind="ExternalInput")
    xv8 = nc.dram_tensor("xv8", [P, S // 512, 2, KT, 2, 512], F8, kind="ExternalInput")
    wq8 = nc.dram_tensor("wq8", [P, 3, KT, 2, E], F8, kind="ExternalInput")
    wk8 = nc.dram_tensor("wk8", [P, 3, KT, 2, E], F8, kind="ExternalInput")
    wv8 = nc.dram_tensor("wv8", [P, 3, KT, 2, E], F8, kind="ExternalInput")
    wo = nc.dram_tensor("wo", [P, KE, D], BF16, kind="ExternalInput")
    bqT = nc.dram_tensor("bqT", [P, KE], F32, kind="ExternalInput")
    bkT = nc.dram_tensor("bkT", [P, KE], F32, kind="ExternalInput")
    bvw = nc.dram_tensor("bvw", [1, E], BF16, kind="ExternalInput")
    ones_c = nc.dram_tensor("ones_c", [1, P], BF16, kind="ExternalInput")
    ident = nc.dram_tensor("ident", [P, P], BF16, kind="ExternalInput")
    maskT = nc.dram_tensor("maskT", [P, SK, S], BF16, kind="ExternalInput")
    out = nc.dram_tensor("out", [P, S // P, D], BF16, kind="ExternalOutput")

    with tile.TileContext(nc) as tc:
        with (
            tc.tile_pool(name="persist", bufs=1) as persist,
            tc.tile_pool(name="ax", bufs=4) as ax,
            tc.tile_pool(name="bm", bufs=8) as bm,
            tc.tile_pool(name="be", bufs=5) as be,
            tc.tile_pool(name="bp", bufs=2) as bp,
            tc.tile_pool(name="bxa", bufs=2) as bxa,
            tc.tile_pool(name="bxt", bufs=1) as bxt,
            tc.tile_pool(name="bo", bufs=3) as bo_,
            tc.tile_pool(name="brc", bufs=4) as brc,
            tc.tile_pool(name="psS", bufs=2, space="PSUM") as psS,
            tc.tile_pool(name="psV", bufs=2, space="PSUM") as psV,
            tc.tile_pool(name="psO", bufs=2, space="PSUM") as psO,
        ):
            qT_sb = persist.tile([P, KE, S], BF16)
            kT_sb = persist.tile([P, KE, S], BF16)
            v_sb = persist.tile([P, SK, H, DK1], BF16)
            wq_sb = persist.tile([P, 3, KT, 2, E], F8)
            wk_sb = persist.tile([P, 3, KT, 2, E], F8)
            wv_sb = persist.tile([P, 3, KT, 2, E], F8)
            wo_sb = persist.tile([P, KE, D], BF16)
            bq_sb = persist.tile([P, KE], F32)
            bk_sb = persist.tile([P, KE], F32)
            bvw_sb = persist.tile([1, E], BF16)
            ones_sb = persist.tile([1, P], BF16)
            id_sb = persist.tile([P, P], BF16)
            nc.gpsimd.memset(v_sb[:, :, :, DK:DK1], 1.0)

            TERMS = [(0, 0), (1, 1), (0, 2)]  # (x ver, w ver): xh@wh + xl4@wh4 + xh@wl

            # ---------------- emission helpers ----------------
            x_tiles = {}

            def issue_x(which, c):
                xd = {"q": xq8, "k": xk8, "v": xv8}[which]
                xt = ax.tile([P, 2, KT, 2, CS], F8, tag="x", name=f"x{which}{c}")
                nc.gpsimd.dma_start(out=xt[:], in_=xd[:, c])
                x_tiles[(which, c)] = xt

            def emit_q(c, which):
                w_sb, b_sb, t_sb = {
                    "q": (wq_sb, bq_sb, qT_sb),
                    "k": (wk_sb, bk_sb, kT_sb),
                }[which]
                ssl = slice(c * CS, (c + 1) * CS)
                xt = x_tiles.pop((which, c))
                for et in range(KE):
                    esl = slice(et * P, (et + 1) * P)
                    ps = psV.tile([P, CS], F32, tag="v")
                    n = 0
                    for xv, wv in TERMS:
                        for kt in range(KT):
                            nc.tensor.matmul(
                                ps[:], w_sb[:, wv, kt, :, esl], xt[:, xv, kt, :, :],
                                start=(n == 0), stop=(n == 3 * KT - 1),
                                perf_mode=DR,
                            )
                            n += 1
                    nc.vector.tensor_scalar(
                        out=t_sb[:, et, ssl], in0=ps[:],
                        scalar1=1.0 / WS, scalar2=b_sb[:, et : et + 1],
                        op0=MULT, op1=ADD,
                    )

            xv_tiles = {}

            def emit_v(c, st):
                xt = x_tiles[("v", c)]
                stg = c * (CS // P) + st
                psl = slice(st * P, (st + 1) * P)
                ps = psO.tile([P, E], F32, tag="o2")
                n = 0
                for xv, wv in TERMS:
                    for kt in range(KT):
                        nc.tensor.matmul(
                            ps[:], xt[:, xv, kt, :, psl], wv_sb[:, wv, kt, :, :],
                            start=(n == 0), stop=False, perf_mode=DR,
                        )
                        n += 1
                nc.tensor.matmul(ps[:], ones_sb[:], bvw_sb[:], start=False, stop=True)
                nc.vector.tensor_scalar(
                    out=v_sb[:, stg, :, 0:DK],
                    in0=ps[:].rearrange("p (h d) -> p h d", h=H),
                    scalar1=1.0 / WS, scalar2=None, op0=MULT,
                )

            def mk_pv(pTh, h, qsub, xatt_t):
                def f():
                    xa = psV.tile([P, DK1], F32, tag="v")
                    qsl = slice(qsub * P, (qsub + 1) * P)
                    for sk in range(SK):
                        nc.tensor.matmul(
                            xa[:], pTh[:, sk, qsl], v_sb[:, sk, h, :],
                            start=(sk == 0), stop=(sk == SK - 1),
                        )
                    rec = brc.tile([P, 1], F32, tag="rc")
                    nc.vector.reciprocal(rec[:], xa[:, DK:DK1])
                    nc.vector.tensor_scalar(
                        out=xatt_t[:, qsub, h * DK : (h + 1) * DK],
                        in0=xa[:, 0:DK], scalar1=rec[:], scalar2=None, op0=MULT,
                    )
                    return SK * DK1 + 500
                return f

            def mk_tr(xatt_t, xaT_t, qsub, pool=None, ptag="o2", split_act=False):
                def f():
                    for et in range(KE):
                        pt = (pool or psO).tile([P, P], BF16, tag=ptag, name="pt")
                        nc.tensor.transpose(
                            pt[:], xatt_t[:, qsub, et * P : (et + 1) * P], id_sb[:]
                        )
                        dst = xaT_t[:, et, qsub * P : (qsub + 1) * P]
                        if split_act and et == 0:
                            nc.scalar.copy(out=dst, in_=pt[:])
                        else:
                            nc.vector.tensor_copy(dst, pt[:])
                    return 2 * P + 400
                return f

            def mk_op(xaT_t, cq, qsub, use_act=False, dn1_psv=False):
                def f():
                    ot = bo_.tile([P, D], BF16, tag="o")
                    qsl = slice(qsub * P, (qsub + 1) * P)
                    for dn in range(D // 512):
                        if dn1_psv and dn == 1:
                            po = psV.tile([P, 512], F32, tag="v", name="po1")
                        else:
                            po = psO.tile([P, 512], F32, tag="o2")
                        dsl = slice(dn * 512, (dn + 1) * 512)
                        for et in range(KE):
                            nc.tensor.matmul(
                                po[:], xaT_t[:, et, qsl], wo_sb[:, et, dsl],
                                start=(et == 0), stop=(et == KE - 1),
                            )
                        if use_act and dn == 0:
                            nc.scalar.copy(out=ot[:, dsl], in_=po[:])
                        else:
                            nc.vector.tensor_copy(ot[:, dsl], po[:])
                    if use_act:
                        nc.sync.dma_start(out=out[:, cq * QS + qsub, :], in_=ot[:])
                    else:
                        nc.gpsimd.dma_start(out=out[:, cq * QS + qsub, :], in_=ot[:])
                    return 2 * D + 600
                return f

            # ---------------- schedule ----------------
            pending = deque()  # (tag, cost_estimate, closure)

            def pull(budget):
                while pending and budget > 0:
                    tag, cost, f = pending.popleft()
                    r = f()
                    budget -= cost if r is None else r

            def flush(tag_needed):
                while any(t == tag_needed for t, _, _ in pending):
                    t, cost, f = pending.popleft()
                    f()

            # PE warmup: ramp the p-state to full clock before the first
            # projection data lands (dummy matmuls on a zeroed tile)
            warm_sb = persist.tile([1, 512], BF16, name="warm_sb")
            nc.gpsimd.memset(warm_sb[:], 0.0)
            for wi in range(12):
                ps_w = psS.tile([P, 512], F32, tag="s", name="wps")
                nc.tensor.matmul(
                    ps_w[:], warm_sb[:, 0:P], warm_sb[:], start=True, stop=True
                )

            # head: minimal DMA chain to the first scores: wq,xq0 / wk,xk0 / xq1
            # (head x chunks on the SP queue — they carry no WAR waits; later
            # chunks go through the Pool queue whose waits don't block issue)
            def issue_x_sp(which, c, split=False):
                xd = {"q": xq8, "k": xk8, "v": xv8}[which]
                xt = ax.tile([P, 2, KT, 2, CS], F8, tag="x", name=f"x{which}{c}")
                if split:
                    nc.sync.dma_start(out=xt[:, 0], in_=xd[:, c, 0])
                    nc.sync.dma_start(out=xt[:, 1], in_=xd[:, c, 1])
                else:
                    nc.sync.dma_start(out=xt[:], in_=xd[:, c])
                x_tiles[(which, c)] = xt

            mask_pairs = {}

            def mask_dma(cq, pair):
                if pair in mask_pairs and mask_pairs[pair][1] == cq:
                    return
                mt = bm.tile([P, 2, CQ], BF16, tag="m", name=f"m{cq}_{pair}")
                nc.sync.dma_start(
                    out=mt[:],
                    in_=maskT[:, 2 * pair : 2 * pair + 2, cq * CQ : (cq + 1) * CQ],
                )
                mask_pairs[pair] = (mt, cq)

            def mask_ap(sk):
                return mask_pairs[sk // 2][0][:, sk % 2, :]

            nc.sync.dma_start(out=wq_sb[:], in_=wq8[:])
            nc.sync.dma_start(out=bq_sb[:], in_=bqT[:])
            issue_x_sp("q", 0)
            nc.sync.dma_start(out=wk_sb[:], in_=wk8[:])
            nc.sync.dma_start(out=bk_sb[:], in_=bkT[:])
            issue_x_sp("k", 0)
            mask_dma(0, 0)
            issue_x_sp("q", 1)
            issue_x_sp("k", 1)
            emit_q(0, "q")
            emit_q(0, "k")
            for wi in range(8):
                ps_w2 = psS.tile([P, 512], F32, tag="s", name="wps2")
                nc.tensor.matmul(
                    ps_w2[:], warm_sb[:, 0:P], warm_sb[:], start=True, stop=True
                )
            emit_q(1, "q")
            nc.sync.dma_start(out=wv_sb[:], in_=wv8[:])
            nc.sync.dma_start(out=bvw_sb[:], in_=bvw[:])
            nc.sync.dma_start(out=ones_sb[:], in_=ones_c[:])

            xatt_tiles = {}
            xaT_tiles = {}

            for cq in range(NCQ):
                xatt_t = bxa.tile([P, QS, E], BF16, tag="xatt")
                xatt_tiles[cq] = xatt_t
                if cq > 0:
                    mask_dma(cq, 0)
                for h in range(H):
                    half, ke = h & 1, h >> 1
                    pdsl = slice(64 * half, 64 * half + 64)
                    pTh = bp.tile([P, SK, CQ], BF16, tag="pT", name=f"pT{cq}_{h}")
                    if h == 0:
                        # sk0/sk1 emitted piece-wise: the lower halves need
                        # only the first q chunk of this cq block, so the exp
                        # stream starts before the second chunk is projected
                        mask_dma(cq, 1)
                        ss2 = [psS.tile([P, CQ], F32, tag="s", name=f"ss2_{i}")
                               for i in range(2)]
                        et2 = [be.tile([P, CQ], BF16, tag="e", name=f"et2_{i}")
                               for i in range(2)]
                        for piece in range(2):
                            if cq > 0 and piece == 1:
                                pull(4000)  # drain the deferred q-projection
                            for sk in range(2):
                                ss_, et_ = ss2[sk], et2[sk]
                                psl_ = slice(piece * 512, (piece + 1) * 512)
                                gsl_ = slice(cq * CQ + piece * 512,
                                             cq * CQ + (piece + 1) * 512)
                                nc.tensor.matmul(
                                    ss_[:, psl_],
                                    kT_sb[pdsl, ke, sk * P : (sk + 1) * P],
                                    qT_sb[pdsl, ke, gsl_],
                                    start=True, stop=True,
                                )
                                nc.scalar.activation(et_[:, psl_], ss_[:, psl_], EXP)
                                nc.vector.tensor_mul(
                                    pTh[:, sk, psl_], et_[:, psl_],
                                    mask_ap(sk)[:, psl_],
                                )
                        sk_range = range(2, SK)
                    else:
                        sk_range = range(SK)
                    for sk in sk_range:
                        if cq == 0 and h == 0:
                            if sk % 4 == 0 and sk > 0:
                                emit_q(sk // 4, "k")
                            if sk == 2:
                                issue_x("k", 2)
                            elif sk == 3:
                                issue_x("k", 3)
                            elif sk in (5, 7, 9):
                                issue_x("v", (sk - 5) // 2)
                            elif sk == 13:
                                issue_x("v", 3)
                        if cq == 0 and h == 1 and sk == 0:
                            nc.sync.dma_start(out=id_sb[:], in_=ident[:])
                            nc.sync.dma_start(out=wo_sb[:], in_=wo[:])
                        if cq == 0 and h == 2 and sk in (0, 2):
                            issue_x("q", 2 + sk // 2)
                        if h == 0 and sk + 2 < SK:
                            mask_dma(cq, (sk + 2) // 2)
                        ss = psS.tile([P, CQ], F32, tag="s")
                        et_t = be.tile([P, CQ], BF16, tag="e")
                        for n2 in range(CQ // 512):
                            nsl = slice(n2 * 512, (n2 + 1) * 512)
                            gsl = slice(cq * CQ + n2 * 512, cq * CQ + (n2 + 1) * 512)
                            nc.tensor.matmul(
                                ss[:, nsl], kT_sb[pdsl, ke, sk * P : (sk + 1) * P],
                                qT_sb[pdsl, ke, gsl], start=True, stop=True,
                            )
                        nc.scalar.activation(et_t[:], ss[:], EXP)
                        nc.vector.tensor_mul(pTh[:, sk, :], et_t[:], mask_ap(sk))
                        if cq == 0 and h == 0:
                            pull(500 if sk > 9 else 200)
                        else:
                            pull(2400 if sk < 6 else 1200)
                    # post-head work
                    if cq == 0 and h == 0:
                        for c in range(NCS):
                            for st in range(CS // P):
                                pending.append(
                                    ("v", 1900, (lambda c=c, st=st: emit_v(c, st)))
                                )
                    if cq == 0 and h == 2:
                        pending.append(("proj", 3400, lambda: emit_q(2, "q")))
                        pending.append(("proj2", 3400, lambda: emit_q(3, "q")))
                    if h < H - 1:
                        for qsub in range(QS):
                            pending.append(("pv", SK * DK1 + 500, mk_pv(pTh, h, qsub, xatt_t)))
                    else:
                        # stagger PV with transpose/out-proj so the per-qsub
                        # chains pipeline through the 2-slot psum pools
                        xaT_t = bxt.tile([P, KE, CQ], BF16, tag="xaT")
                        xaT_tiles[cq] = xaT_t
                        last = cq == NCQ - 1
                        tr_pool, tr_tag = (psS, "s") if last else (None, "o2")
                        lag = 2 if last else 1
                        if last:
                            for qsub in range(QS):
                                pending.append(("pv", SK * DK1 + 500, mk_pv(pTh, h, qsub, xatt_t)))
                                if qsub >= lag:
                                    j = qsub - lag
                                    pending.append(("tr", 2 * P + 400, mk_tr(xatt_t, xaT_t, j, tr_pool, tr_tag, last)))
                                    pending.append(("op", 2 * D + 600, mk_op(xaT_t, cq, j, last, last)))
                            for j in range(QS - lag, QS):
                                pending.append(("tr", 2 * P + 400, mk_tr(xatt_t, xaT_t, j, tr_pool, tr_tag, last)))
                                pending.append(("op", 2 * D + 600, mk_op(xaT_t, cq, j, last, last)))
                        else:
                            # defer the out-projections behind all transposes:
                            # their DVE copies otherwise oversubscribe DVE in
                            # the next chunk's first head and stall ACT
                            for qsub in range(QS):
                                pending.append(("pv", SK * DK1 + 500, mk_pv(pTh, h, qsub, xatt_t)))
                                if qsub >= 1:
                                    j = qsub - 1
                                    pending.append(("tr", 2 * P + 400, mk_tr(xatt_t, xaT_t, j, tr_pool, tr_tag, False)))
                            pending.append(("tr", 2 * P + 400, mk_tr(xatt_t, xaT_t, QS - 1, tr_pool, tr_tag, False)))
                            for j in range(QS):
                                pending.append(("op", 2 * D + 600, mk_op(xaT_t, cq, j, False, False)))
                if cq == 0:
                    flush("proj")
            while pending:
                _, _, f = pending.popleft()
                f()

    _split_multiwait(nc, 1)
    return nc


# ---------------------------------------------------------------- host side

B, S_FULL, D_FULL, H_FULL = 2, 2048, 1024, 16
DK_FULL = D_FULL // H_FULL
N_CORES = 8
GROUPS = N_CORES // B   # head-groups per batch
EG = D_FULL // GROUPS   # e-columns per core

_NC_CACHE = {}


def _get_program():
    if "full" not in _NC_CACHE:
        _NC_CACHE["full"] = build_program(D=D_FULL, S=S_FULL, E=EG, DK=DK_FULL)
    return _NC_CACHE["full"]


def _f8(a):
    import ml_dtypes

    return a.astype(ml_dtypes.float8_e4m3fn)


def _bf(a):
    import ml_dtypes

    return np.ascontiguousarray(a, dtype=np.float32).astype(ml_dtypes.bfloat16)


def _prep_x(aT, ncols):
    """[1024, ncols] f32 -> [128, ncols//512, 2, 4, 2, 512] fp8 (hi, lo*4)
    with d = 256*kt + 128*u + p; chunk-major for big-descriptor DMA."""
    a = np.ascontiguousarray(aT, dtype=np.float32)
    hi = _f8(a)
    lo4 = _f8((a - hi.astype(np.float32)) * 4.0)
    v = np.stack([hi, lo4], axis=0)
    v = v.reshape(2, 4, 2, 128, ncols).transpose(3, 0, 1, 2, 4)
    v = v.reshape(128, 2, 4, 2, ncols // 512, 512).transpose(0, 4, 1, 2, 3, 5)
    return np.ascontiguousarray(v)


def _prep_w(aT, ncols):
    """[1024, ncols] f32 (pre-scaled by WS) -> [128, 3, 4, 2, ncols] fp8
    versions (wh, wh/4, wl)."""
    a = np.ascontiguousarray(aT, dtype=np.float32)
    wh = _f8(a)
    whf = wh.astype(np.float32)
    wh4 = _f8(whf / 4.0)
    wl = _f8(a - whf)
    v = np.stack([wh, wh4, wl], axis=0)
    v = v.reshape(3, 4, 2, 128, ncols).transpose(3, 0, 1, 2, 4)
    return np.ascontiguousarray(v)


LAST_RES = None


def kernel(query, key, value, softmask, Wq, bq, Wk, bk, Wv, bv, Wo, bo, _trace=False):
    global LAST_RES
    from concourse.bass_utils import run_bass_kernel_spmd

    nc = _get_program()
    scale = np.float32(1.0 / math.sqrt(DK_FULL))

    x_cache = {}
    for b in range(B):
        x_cache[b] = (
            _prep_x(np.asarray(query[b], np.float32).T, S_FULL),
            _prep_x(np.asarray(key[b], np.float32).T, S_FULL),
            _prep_x(np.asarray(value[b], np.float32).T, S_FULL),
            np.ascontiguousarray(
                _bf(np.asarray(softmask[b], np.float32).T + 1e-30)
                .reshape(S_FULL // 128, 128, S_FULL)
                .transpose(1, 0, 2)
            ),
        )

    ident = _bf(np.eye(128, dtype=np.float32))
    ones_c = _bf(np.ones((1, 128), np.float32))

    in_maps = []
    for c in range(N_CORES):
        b, g = c // GROUPS, c % GROUPS
        es = slice(g * EG, (g + 1) * EG)
        xq8, xk8, xv8, mT = x_cache[b]
        m = {
            "xq8": xq8, "xk8": xk8, "xv8": xv8, "maskT": mT,
            "wq8": _prep_w(Wq[es, :].T * (scale * WS), EG),
            "wk8": _prep_w(Wk[es, :].T * WS, EG),
            "wv8": _prep_w(Wv[es, :].T * WS, EG),
            "wo": np.ascontiguousarray(
                _bf(Wo[:, es].T).reshape(EG // 128, 128, D_FULL).transpose(1, 0, 2)
            ),
            "bqT": np.ascontiguousarray(
                (np.asarray(bq[es], np.float32) * scale).reshape(EG // 128, 128).T
            ),
            "bkT": np.ascontiguousarray(
                np.asarray(bk[es], np.float32).reshape(EG // 128, 128).T
            ),
            "bvw": _bf(np.asarray(bv[es], np.float32)[None, :] * WS),
            "ones_c": ones_c,
            "ident": ident,
        }
        in_maps.append(m)

    res = run_bass_kernel_spmd(
        nc, in_maps, core_ids=list(range(N_CORES)), trace=_trace
    )
    LAST_RES = res

    outp = np.zeros((B, S_FULL, D_FULL), dtype=np.float32)
    for c in range(N_CORES):
        b = c // GROUPS
        o = res.results[c]["out"].astype(np.float32)  # [128, 16, D]
        outp[b] += o.transpose(1, 0, 2).reshape(S_FULL, D_FULL)
    outp += np.asarray(bo, dtype=np.float32)[None, None, :]
    return outp
